# revision 58
# baseline (speedup 1.0000x reference)
"""Trainium2 Bass kernel for nn_AFM (attention-modulated 3x3 conv).

Math (per batch):
    ratio = MLP_a2(mean_hw(x))                       # [9]
    bias3 = MLP_a3(mean_hw(x))                       # [64]
    y[m,p] = sum_{c,t} W[m,c,t] * x[c, p+delta_t] * (atw1[t,p]*ratio[t]) + bias3[m]

Strategy: data-parallel over batch (8 cores, 1 batch each, no collectives).
Per core, fp16 compute:
  - attention (with conv zero-pad validity baked in as zeros, guard columns)
    ships as an f16 input `ag`; x ships as guarded f16 `xg`.
  - ratio is folded into the conv weights on device (so atw1 is used raw).
  - taps are processed in pairs sharing one 128-row contraction:
      R_pair[(c,t), q] = x[c, q+u_t] * ag[t, q - o_pair]
    built by DVE f16 tensor_tensor (2x mode; the lone ninth tap runs on
    GpSimd), with per-pair attention replica tiles produced by DRAM->SBUF
    broadcast DMA (partition step 0) spread over both HWDGE queues.
  - seam-split tiling: R chunks are exact-width (chunk 0 carries the left
    halo); matmul rhs reads split at chunk seams, relying on PSUM
    has_written semantics (first writer overwrites, later ones accumulate),
    so no halo columns are recomputed or re-broadcast. Out-of-range reads
    at the right edge are dropped: their attention is masked to zero.
  - 5 PSUM-accumulated f16 matmuls per 512-pixel tile compute y directly;
    ScalarE evacuates PSUM with the a3-bias add fused; y ships as f16 and
    the host upcasts.
"""

import numpy as np
from contextlib import ExitStack

import concourse.bass as bass
import concourse.tile as tile
from concourse import bacc, mybir
from concourse.bass_utils import run_bass_kernel_spmd

# permuted tap order: rows are taps [0,2, 3,5, 6,8, 1,4, 7] so that each
# matmul group's two taps sit on adjacent rows of `ag`
PERM = [0, 2, 3, 5, 6, 8, 1, 4, 7]
# groups: (row0, row1|None, o = rhs pixel offset, u = upper-half x shift)
GROUPS = [
    (0, 1, -129, 2),
    (2, 3, -1, 2),
    (4, 5, 127, 2),
    (6, 7, -128, 128),
    (8, None, 128, 0),
]
HH = 128
WW = 128
P = HH * WW           # 16384 pixels
CIN = 64
COUT = 64
GUARD = 264           # zero guard columns on xg/ag (>= 132 + 129)
GL = 132              # per-chunk halo for rhs offsets (|o| <= 129)
CH = 2048             # pixels per chunk
NCH = P // CH
WCH = CH + 2 * GL     # 2312 columns per R/ar chunk tile
WX = P + 2 * GUARD    # 16912
AR_VIA_PE = ()    # pair-groups whose attention replicas come from a
                      # PE broadcast-matmul + ScalarE evac instead of DMA

# packed small-weight blob layout (f32, [128, BLOB_W]); columns:
#   wl: 5 groups x 64      -> 0..320     (rows 0..127)
#   a2w1T [64,9]           -> 320..329   (rows 0..63)
#   a2b1 [9,1]             -> 329..330   (rows 0..8)
#   a2w2r [9,640]          -> 330..970   (rows 0..8)
#   a2b2r [128,5]          -> 970..975
#   a3w1T [64,64]          -> 975..1039  (rows 0..63)
#   a3b1 [64,1]            -> 1039..1040
#   a3w2T [64,64]          -> 1040..1104
#   a3b2 [64,1]            -> 1104..1105
BLOB_W = 1105
C_WL, C_A2W1, C_A2B1, C_A2W2R, C_A2B2R = 0, 320, 329, 330, 970
C_A3W1, C_A3B1, C_A3W2, C_A3B2 = 975, 1039, 1040, 1104

_CACHE = {}


def _build_nc():
    f32, f16 = mybir.dt.float32, mybir.dt.float16
    AF = mybir.ActivationFunctionType
    OP = mybir.AluOpType

    nc = bacc.Bacc("TRN2", target_bir_lowering=False, debug=False,
                   enable_asserts=True, num_devices=8)
    xg = nc.dram_tensor("xg", [CIN, WX], f16, kind="ExternalInput").ap()
    ag = nc.dram_tensor("ag", [9, WX], f16, kind="ExternalInput").ap()
    wb = nc.dram_tensor("wb", [128, BLOB_W], f32, kind="ExternalInput").ap()
    if AR_VIA_PE:
        sg = nc.dram_tensor("sg", [34, 128], mybir.dt.float16,
                            kind="ExternalInput").ap()
    y = nc.dram_tensor("y", [COUT, P], f16, kind="ExternalOutput").ap()

    # round-robin DMA issue engines (separate queues)
    def dq(i):
        return [nc.sync, nc.scalar][i % 2]

    with tile.TileContext(nc) as tc:
        with ExitStack() as ctx:
            sing = ctx.enter_context(tc.tile_pool(name="sing", bufs=1))
            arp = ctx.enter_context(tc.tile_pool(name="arp", bufs=2))
            rp = ctx.enter_context(tc.tile_pool(name="rp", bufs=3))
            trp = ctx.enter_context(tc.tile_pool(name="trp", bufs=1))
            psy = ctx.enter_context(tc.tile_pool(name="psy", bufs=2, space="PSUM"))
            psm = ctx.enter_context(tc.tile_pool(name="psm", bufs=2, space="PSUM"))
            psb = ctx.enter_context(tc.tile_pool(name="psb", bufs=2, space="PSUM"))

            # small-weight blob
            wbt = sing.tile([128, BLOB_W], f32)
            nc.sync.dma_start(out=wbt, in_=wb)

            # attention rows in SBUF (PE broadcast source): one tile, each
            # PE-pair's 2 rows at a legal matmul base partition (0, 32)
            atg, sel = {}, {}
            if AR_VIA_PE:
                atgt = sing.tile([34, WX], f16)
                selt = sing.tile([34, 128], f16)
                nc.sync.dma_start(out=selt, in_=sg)
                for j, g in enumerate(AR_VIA_PE):
                    r0 = GROUPS[g][0]
                    nc.scalar.dma_start(out=atgt[32 * j:32 * j + 2, :],
                                        in_=ag[r0:r0 + 2, :])
                    atg[g] = atgt[32 * j:32 * j + 2, :]
                    sel[g] = selt[32 * j:32 * j + 2, :]

            # x (f16, guarded) -> lower half of XXL, loaded in chunks; the
            # g-mean accumulation runs per chunk, alternating ScalarE (f16
            # copy with accum_out) and VectorE (reduce); the +2-shifted upper
            # half is built in per-chunk pieces so early chunks unblock fast
            XXL = sing.tile([128, WX], f16)
            gparts = sing.tile([CIN, NCH], f32)
            nc.scalar.dma_start(out=XXL[0:64, 0:GUARD], in_=xg[:, 0:GUARD])
            nc.gpsimd.dma_start(out=XXL[0:64, GUARD + P:], in_=xg[:, GUARD + P:])
            for k in range(NCH):
                c0 = GUARD + k * CH
                dq(k).dma_start(out=XXL[0:64, c0:c0 + CH], in_=xg[:, c0:c0 + CH])
                if k % 2 == 0:
                    trash = trp.tile([CIN, CH], f16, tag="trash")
                    nc.scalar.activation(out=trash[:], in_=XXL[0:64, c0:c0 + CH],
                                         func=AF.Copy,
                                         accum_out=gparts[:, k:k + 1])
                else:
                    nc.vector.tensor_reduce(gparts[:, k:k + 1],
                                            XXL[0:64, c0:c0 + CH],
                                            axis=mybir.AxisListType.X, op=OP.add)
                if k > 0:
                    w0 = (k - 1) * CH + GUARD - GL
                    nc.vector.tensor_copy(out=XXL[64:128, w0:w0 + WCH],
                                          in_=XXL[0:64, w0 + 2:w0 + 2 + WCH])
            w0 = (NCH - 1) * CH + GUARD - GL
            nc.vector.tensor_copy(out=XXL[64:128, w0:w0 + WCH],
                                  in_=XXL[0:64, w0 + 2:w0 + 2 + WCH])

            Y16 = sing.tile([COUT, P], f16)   # staged f16 output

            gsum = sing.tile([CIN, 1], f32)
            nc.vector.tensor_reduce(gsum[:], gparts[:], axis=mybir.AxisListType.X,
                                    op=OP.add)

            # attention2: h1 = relu(g @ a2w1.T / P + b1); ratio replicated per
            # group via host-widened a2w2 columns
            h1ps = psm.tile([9, 1], f32, tag="mlp")
            nc.tensor.matmul(h1ps[:], wbt[0:64, C_A2W1:C_A2W1 + 9], gsum[:],
                             start=True, stop=True)
            h1r = sing.tile([9, 1], f32)
            nc.scalar.activation(out=h1r[:], in_=h1ps[:], func=AF.Relu,
                                 bias=wbt[0:9, C_A2B1:C_A2B1 + 1], scale=1.0 / P)
            rrep = []
            for g in range(5):
                rps = psm.tile([128, 1], f32, tag="mlp")
                nc.tensor.matmul(
                    rps[:], wbt[0:9, C_A2W2R + g * 128:C_A2W2R + (g + 1) * 128],
                    h1r[:], start=True, stop=True)
                rr = sing.tile([128, 1], f32, tag=f"rr{g}")
                nc.scalar.activation(out=rr[:], in_=rps[:], func=AF.Identity,
                                     bias=wbt[:, C_A2B2R + g:C_A2B2R + g + 1],
                                     scale=1.0)
                rrep.append(rr)

            # attention3 bias: bias3 = relu(g @ a3w1.T / P + b1) @ a3w2.T + b2
            h3ps = psm.tile([COUT, 1], f32, tag="mlp")
            nc.tensor.matmul(h3ps[:], wbt[0:64, C_A3W1:C_A3W1 + 64], gsum[:],
                             start=True, stop=True)
            h3r = sing.tile([COUT, 1], f32)
            nc.scalar.activation(out=h3r[:], in_=h3ps[:], func=AF.Relu,
                                 bias=wbt[0:64, C_A3B1:C_A3B1 + 1], scale=1.0 / P)
            b3ps = psm.tile([COUT, 1], f32, tag="mlp")
            nc.tensor.matmul(b3ps[:], wbt[0:64, C_A3W2:C_A3W2 + 64], h3r[:],
                             start=True, stop=True)
            bias3 = sing.tile([COUT, 1], f32)
            nc.scalar.activation(out=bias3[:], in_=b3ps[:], func=AF.Identity,
                                 bias=wbt[0:64, C_A3B2:C_A3B2 + 1], scale=1.0)

            # fold ratio into conv weights: wf_g = wl_g * rrep_g  (f16)
            wf = []
            for g, (r0, r1, o, u) in enumerate(GROUPS):
                parts = 128 if r1 is not None else 64
                t = sing.tile([parts, COUT], f16, tag=f"wf{g}")
                nc.vector.tensor_scalar(t[:], wbt[0:parts, C_WL + g * 64:
                                                 C_WL + (g + 1) * 64],
                                        rrep[g][0:parts, :], None, OP.mult)
                wf.append(t)

            # main pixel-chunk loop, seam-split tiling over non-uniform R
            # chunks: chunk 0 carries the left halo, the final 2048 pixels
            # are split 1536+512 so the last dependency in the drain tail is
            # tiny. Matmul rhs reads split at chunk seams, relying on PSUM
            # has_written semantics (first writer overwrites, later parts
            # accumulate). Right-edge reads past P are dropped: their
            # attention is masked to zero (out-of-image taps).
            BND = [-GL] + [CH * (k + 1) for k in range(NCH)]
            NRC = len(BND) - 1          # 9 R chunks

            def rloc(q):
                for ci in range(NRC):
                    if q < BND[ci + 1]:
                        return ci, q - BND[ci]
                raise AssertionError(q)

            # REQ[m]: R chunk whose completion unlocks output chunk m fully
            REQ = [rloc((m + 1) * CH + 127)[0] if (m + 1) * CH + 127 < P
                   else NRC - 1 for m in range(NCH)]

            rtiles = []

            def emit_matmuls(m, req_ci):
                for half in range(2):
                    yps = psy.tile([COUT, 1024], f32)
                    parts = []      # (s, ci, col, pcol, w, g)
                    for g, (r0, r1, o, u) in enumerate(GROUPS):
                        for s in range(2):
                            q0 = m * CH + half * 1024 + s * 512 + o
                            left = 512
                            pcol = s * 512
                            while left > 0:
                                if q0 >= P:
                                    break   # masked-zero region
                                ci, c1 = rloc(q0)
                                w = min(left, BND[ci + 1] - q0)
                                parts.append((s, ci, c1, pcol, w, g))
                                q0 += w
                                pcol += w
                                left -= w
                    # defer parts needing the just-built newest chunk so the
                    # rest can run while its inputs are still in flight
                    parts = ([p for p in parts if p[1] < req_ci] +
                             [p for p in parts if p[1] >= req_ci])
                    first = {}
                    lastp = {}
                    for i, p in enumerate(parts):
                        first.setdefault(p[0], i)
                        lastp[p[0]] = i
                    for i, (s, ci, c1, pcol, w, g) in enumerate(parts):
                        nc.tensor.matmul(
                            yps[:, pcol:pcol + w], wf[g][:],
                            rtiles[ci][g][:, c1:c1 + w],
                            start=first[s] == i, stop=lastp[s] == i)
                    off = m * CH + half * 1024
                    nc.scalar.activation(out=Y16[:, off:off + 1024], in_=yps[:],
                                         func=AF.Identity, bias=bias3[:],
                                         scale=1.0)

            dmai = 0
            for ci in range(NRC):
                qlo, qhi = BND[ci], BND[ci + 1]
                wk = qhi - qlo
                col0 = qlo + GUARD          # XXL column of the chunk start

                # attention replica tiles (DRAM->SBUF broadcast DMA)
                arts = []
                for g, (r0, r1, o, u) in enumerate(GROUPS):
                    s0 = qlo - o + GUARD
                    if g == 3:
                        # one DMA: both halves column-concatenated (base
                        # partition 0 for both TT reads)
                        a01 = arp.tile([64, 2 * wk], f16, tag="ar3")
                        row = ag[r0:r0 + 1, s0:s0 + wk]
                        src = bass.AP(tensor=row.tensor, offset=row.offset,
                                      ap=[[0, 64], [WX, 2], list(row.ap)[-1]])
                        nc.sync.dma_start(out=a01[:], in_=src)
                        arts.append(a01)
                        continue
                    if r1 is None:
                        ar8 = arp.tile([64, wk], f16, tag="ar4")
                        row = ag[r0:r0 + 1, s0:s0 + wk]
                        src = bass.AP(tensor=row.tensor, offset=row.offset,
                                      ap=[[0, 64], list(row.ap)[-1]])
                        nc.sync.dma_start(out=ar8[:], in_=src)
                        arts.append(ar8)
                        continue
                    art = arp.tile([128, wk], f16, tag=f"ar{g}")
                    row = ag[r0:r0 + 1, s0:s0 + wk]
                    src = bass.AP(tensor=row.tensor, offset=row.offset,
                                  ap=[[WX, 2], [0, 64], list(row.ap)[-1]])
                    nc.sync.dma_start(out=art[:], in_=src)
                    arts.append(art)

                # R tiles: f16 tensor_tensor multiplies (2x mode on DVE;
                # the lone half-width tap rides the GpSimd engine)
                rts = []
                for g, (r0, r1, o, u) in enumerate(GROUPS):
                    parts_n = 128 if r1 is not None else 64
                    rt = rp.tile([parts_n, wk], f16, tag=f"r{g}")
                    if g < 3:
                        nc.vector.tensor_mul(rt[:], XXL[:, col0:col0 + wk],
                                             arts[g][:])
                    elif g == 3:
                        nc.vector.tensor_mul(rt[0:64, :],
                                             XXL[0:64, col0:col0 + wk],
                                             arts[g][:, 0:wk])
                        nc.vector.tensor_mul(rt[64:128, :],
                                             XXL[0:64, col0 + 128:col0 + 128 + wk],
                                             arts[g][:, wk:2 * wk])
                    else:
                        eng = nc.vector if ci == NRC - 1 else nc.gpsimd
                        eng.tensor_mul(rt[:], XXL[0:64, col0:col0 + wk],
                                       arts[g][:])
                    rts.append(rt)
                rtiles.append(rts)

                for m in range(NCH):
                    if REQ[m] == ci:
                        emit_matmuls(m, ci)

            # deferred output DMAs: the first piece's dependency chain
            # (evacs through chunk 6) keeps it off the fabric until the
            # attention-replica traffic has drained
            cut = (NCH - 2) * CH + 1024
            cut2 = (NCH - 1) * CH + 1024
            nc.sync.dma_start(out=y[:, 0:cut], in_=Y16[:, 0:cut])
            nc.scalar.dma_start(out=y[:, cut:cut2], in_=Y16[:, cut:cut2])
            nc.sync.dma_start(out=y[:, cut2:P], in_=Y16[:, cut2:P])
    nc.compile()
    return nc


def _host_prep(x, atw1, weight, a2_w1, a2_b1, a2_w2, a2_b2, a3_w1, a3_b1,
               a3_w2, a3_b2):
    """Build per-core input maps (host-side layout/dtype prep only)."""
    b = x.shape[0]
    f16 = np.float16

    # validity mask per permuted row: conv zero-padding baked into attention
    hh = np.arange(P) // WW
    ww = np.arange(P) % WW
    masks = np.empty((9, P), np.bool_)
    for r, t in enumerate(PERM):
        kh, kw = t // 3, t % 3
        masks[r] = ((hh + kh - 1 >= 0) & (hh + kh - 1 < HH) &
                    (ww + kw - 1 >= 0) & (ww + kw - 1 < WW))

    blob = np.zeros((128, BLOB_W), np.float32)
    for g, (r0, r1, o, u) in enumerate(GROUPS):
        t0 = PERM[r0]
        blob[0:64, C_WL + g * 64:C_WL + (g + 1) * 64] = \
            weight[:, :, t0 // 3, t0 % 3].T
        if r1 is not None:
            t1 = PERM[r1]
            blob[64:128, C_WL + g * 64:C_WL + (g + 1) * 64] = \
                weight[:, :, t1 // 3, t1 % 3].T
    blob[0:64, C_A2W1:C_A2W1 + 9] = a2_w1.T
    blob[0:9, C_A2B1] = a2_b1
    for g, (r0, r1, o, u) in enumerate(GROUPS):
        blob[0:9, C_A2W2R + g * 128:C_A2W2R + g * 128 + 64] = \
            a2_w2[PERM[r0]][:, None]
        blob[0:64, C_A2B2R + g] = a2_b2[PERM[r0]]
        if r1 is not None:
            blob[0:9, C_A2W2R + g * 128 + 64:C_A2W2R + (g + 1) * 128] = \
                a2_w2[PERM[r1]][:, None]
            blob[64:128, C_A2B2R + g] = a2_b2[PERM[r1]]
    blob[0:64, C_A3W1:C_A3W1 + 64] = a3_w1.T
    blob[0:64, C_A3B1] = a3_b1
    blob[0:64, C_A3W2:C_A3W2 + 64] = a3_w2.T
    blob[0:64, C_A3B2] = a3_b2

    selh = np.zeros((34, 128), np.float16)
    for j in range(2):
        selh[32 * j, 0:64] = 1.0
        selh[32 * j + 1, 64:128] = 1.0

    in_maps = []
    for i in range(b):
        xr = x[i].reshape(CIN, P)
        xgv = np.zeros((CIN, WX), f16)
        xgv[:, GUARD:GUARD + P] = xr.astype(f16)
        at = atw1[i].reshape(9, P)[PERM]
        agv = np.zeros((9, WX), f16)
        agv[:, GUARD:GUARD + P] = np.where(masks, at, 0.0).astype(f16)
        m = {"xg": xgv, "ag": agv, "wb": blob}
        if AR_VIA_PE:
            m["sg"] = selh
        in_maps.append(m)
    return in_maps


def kernel(**inputs):
    x = np.asarray(inputs["x"], np.float32)
    in_maps = _host_prep(
        x, np.asarray(inputs["atw1"], np.float32),
        np.asarray(inputs["weight"], np.float32),
        np.asarray(inputs["a2_w1"], np.float32),
        np.asarray(inputs["a2_b1"], np.float32),
        np.asarray(inputs["a2_w2"], np.float32),
        np.asarray(inputs["a2_b2"], np.float32),
        np.asarray(inputs["a3_w1"], np.float32),
        np.asarray(inputs["a3_b1"], np.float32),
        np.asarray(inputs["a3_w2"], np.float32),
        np.asarray(inputs["a3_b2"], np.float32),
    )
    if "nc" not in _CACHE:
        _CACHE["nc"] = _build_nc()
    nc = _CACHE["nc"]
    res = run_bass_kernel_spmd(nc, in_maps, core_ids=list(range(8)))
    out = np.stack([res.results[i]["y"].reshape(COUT, HH, WW)
                    for i in range(len(in_maps))])
    return out.astype(np.float32)


# revision 60
# speedup vs baseline: 1.0069x; 1.0069x over previous
"""Trainium2 Bass kernel for nn_AFM (attention-modulated 3x3 conv).

Math (per batch):
    ratio = MLP_a2(mean_hw(x))                       # [9]
    bias3 = MLP_a3(mean_hw(x))                       # [64]
    y[m,p] = sum_{c,t} W[m,c,t] * x[c, p+delta_t] * (atw1[t,p]*ratio[t]) + bias3[m]

Strategy: data-parallel over batch (8 cores, 1 batch each, no collectives).
Per core, fp16 compute:
  - attention (with conv zero-pad validity baked in as zeros, guard columns)
    ships as an f16 input `ag`; x ships as guarded f16 `xg`.
  - ratio is folded into the conv weights on device (so atw1 is used raw).
  - taps are processed in pairs sharing one 128-row contraction:
      R_pair[(c,t), q] = x[c, q+u_t] * ag[t, q - o_pair]
    built by DVE f16 tensor_tensor (2x mode; the lone ninth tap runs on
    GpSimd), with per-pair attention replica tiles produced by DRAM->SBUF
    broadcast DMA (partition step 0) spread over both HWDGE queues.
  - seam-split tiling: R chunks are exact-width (chunk 0 carries the left
    halo); matmul rhs reads split at chunk seams, relying on PSUM
    has_written semantics (first writer overwrites, later ones accumulate),
    so no halo columns are recomputed or re-broadcast. Out-of-range reads
    at the right edge are dropped: their attention is masked to zero.
  - 5 PSUM-accumulated f16 matmuls per 512-pixel tile compute y directly;
    ScalarE evacuates PSUM with the a3-bias add fused; y ships as f16 and
    the host upcasts.
"""

import numpy as np
from contextlib import ExitStack

import concourse.bass as bass
import concourse.tile as tile
from concourse import bacc, mybir
from concourse.bass_utils import run_bass_kernel_spmd

# permuted tap order: rows are taps [0,2, 3,5, 6,8, 1,4, 7] so that each
# matmul group's two taps sit on adjacent rows of `ag`
PERM = [0, 2, 3, 5, 6, 8, 1, 4, 7]
# groups: (row0, row1|None, o = rhs pixel offset, u = upper-half x shift)
GROUPS = [
    (0, 1, -129, 2),
    (2, 3, -1, 2),
    (4, 5, 127, 2),
    (6, 7, -128, 128),
    (8, None, 128, 0),
]
HH = 128
WW = 128
P = HH * WW           # 16384 pixels
CIN = 64
COUT = 64
GUARD = 264           # zero guard columns on xg/ag (>= 132 + 129)
GL = 132              # per-chunk halo for rhs offsets (|o| <= 129)
CH = 2048             # pixels per chunk
NCH = P // CH
WCH = CH + 2 * GL     # 2312 columns per R/ar chunk tile
WX = P + 2 * GUARD    # 16912
AR_VIA_PE = ()    # pair-groups whose attention replicas come from a
                      # PE broadcast-matmul + ScalarE evac instead of DMA

# packed small-weight blob layout (f32, [128, BLOB_W]); columns:
#   wl: 5 groups x 64      -> 0..320     (rows 0..127)
#   a2w1T [64,9]           -> 320..329   (rows 0..63)
#   a2b1 [9,1]             -> 329..330   (rows 0..8)
#   a2w2r [9,640]          -> 330..970   (rows 0..8)
#   a2b2r [128,5]          -> 970..975
#   a3w1T [64,64]          -> 975..1039  (rows 0..63)
#   a3b1 [64,1]            -> 1039..1040
#   a3w2T [64,64]          -> 1040..1104
#   a3b2 [64,1]            -> 1104..1105
BLOB_W = 1105
C_WL, C_A2W1, C_A2B1, C_A2W2R, C_A2B2R = 0, 320, 329, 330, 970
C_A3W1, C_A3B1, C_A3W2, C_A3B2 = 975, 1039, 1040, 1104

_CACHE = {}


def _build_nc():
    f32, f16 = mybir.dt.float32, mybir.dt.float16
    AF = mybir.ActivationFunctionType
    OP = mybir.AluOpType

    nc = bacc.Bacc("TRN2", target_bir_lowering=False, debug=False,
                   enable_asserts=True, num_devices=8)
    xg = nc.dram_tensor("xg", [CIN, WX], f16, kind="ExternalInput").ap()
    ag = nc.dram_tensor("ag", [9, WX], f16, kind="ExternalInput").ap()
    wb = nc.dram_tensor("wb", [128, BLOB_W], f32, kind="ExternalInput").ap()
    if AR_VIA_PE:
        sg = nc.dram_tensor("sg", [34, 128], mybir.dt.float16,
                            kind="ExternalInput").ap()
    y = nc.dram_tensor("y", [COUT, P], f16, kind="ExternalOutput").ap()

    # round-robin DMA issue engines (separate queues)
    def dq(i):
        return [nc.sync, nc.scalar][i % 2]

    with tile.TileContext(nc) as tc:
        with ExitStack() as ctx:
            sing = ctx.enter_context(tc.tile_pool(name="sing", bufs=1))
            arp = ctx.enter_context(tc.tile_pool(name="arp", bufs=2))
            rp = ctx.enter_context(tc.tile_pool(name="rp", bufs=3))
            trp = ctx.enter_context(tc.tile_pool(name="trp", bufs=1))
            psy = ctx.enter_context(tc.tile_pool(name="psy", bufs=2, space="PSUM"))
            psm = ctx.enter_context(tc.tile_pool(name="psm", bufs=2, space="PSUM"))
            psb = ctx.enter_context(tc.tile_pool(name="psb", bufs=2, space="PSUM"))

            # small-weight blob
            wbt = sing.tile([128, BLOB_W], f32)
            nc.sync.dma_start(out=wbt, in_=wb)

            # attention rows in SBUF (PE broadcast source): one tile, each
            # PE-pair's 2 rows at a legal matmul base partition (0, 32)
            atg, sel = {}, {}
            if AR_VIA_PE:
                atgt = sing.tile([34, WX], f16)
                selt = sing.tile([34, 128], f16)
                nc.sync.dma_start(out=selt, in_=sg)
                for j, g in enumerate(AR_VIA_PE):
                    r0 = GROUPS[g][0]
                    nc.scalar.dma_start(out=atgt[32 * j:32 * j + 2, :],
                                        in_=ag[r0:r0 + 2, :])
                    atg[g] = atgt[32 * j:32 * j + 2, :]
                    sel[g] = selt[32 * j:32 * j + 2, :]

            # x (f16, guarded) -> lower half of XXL, loaded in chunks; the
            # g-mean accumulation runs per chunk, alternating ScalarE (f16
            # copy with accum_out) and VectorE (reduce); the +2-shifted upper
            # half is built in per-chunk pieces so early chunks unblock fast
            XXL = sing.tile([128, WX], f16)
            gparts = sing.tile([CIN, NCH], f32)
            nc.scalar.dma_start(out=XXL[0:64, 0:GUARD], in_=xg[:, 0:GUARD])
            nc.gpsimd.dma_start(out=XXL[0:64, GUARD + P:], in_=xg[:, GUARD + P:])
            for k in range(NCH):
                c0 = GUARD + k * CH
                dq(k).dma_start(out=XXL[0:64, c0:c0 + CH], in_=xg[:, c0:c0 + CH])
                if k % 2 == 0:
                    trash = trp.tile([CIN, CH], f16, tag="trash")
                    nc.scalar.activation(out=trash[:], in_=XXL[0:64, c0:c0 + CH],
                                         func=AF.Copy,
                                         accum_out=gparts[:, k:k + 1])
                else:
                    nc.vector.tensor_reduce(gparts[:, k:k + 1],
                                            XXL[0:64, c0:c0 + CH],
                                            axis=mybir.AxisListType.X, op=OP.add)
                if k > 0:
                    w0 = (k - 1) * CH + GUARD - (GL if k == 1 else 0)
                    w1 = k * CH + GUARD
                    nc.vector.tensor_copy(out=XXL[64:128, w0:w1],
                                          in_=XXL[0:64, w0 + 2:w1 + 2])
            w0 = (NCH - 1) * CH + GUARD
            w1 = NCH * CH + GUARD
            nc.vector.tensor_copy(out=XXL[64:128, w0:w1],
                                  in_=XXL[0:64, w0 + 2:w1 + 2])

            Y16 = sing.tile([COUT, P], f16)   # staged f16 output

            gsum = sing.tile([CIN, 1], f32)
            nc.vector.tensor_reduce(gsum[:], gparts[:], axis=mybir.AxisListType.X,
                                    op=OP.add)

            # attention2: h1 = relu(g @ a2w1.T / P + b1); ratio replicated per
            # group via host-widened a2w2 columns
            h1ps = psm.tile([9, 1], f32, tag="mlp")
            nc.tensor.matmul(h1ps[:], wbt[0:64, C_A2W1:C_A2W1 + 9], gsum[:],
                             start=True, stop=True)
            h1r = sing.tile([9, 1], f32)
            nc.scalar.activation(out=h1r[:], in_=h1ps[:], func=AF.Relu,
                                 bias=wbt[0:9, C_A2B1:C_A2B1 + 1], scale=1.0 / P)
            rrep = []
            for g in range(5):
                rps = psm.tile([128, 1], f32, tag="mlp")
                nc.tensor.matmul(
                    rps[:], wbt[0:9, C_A2W2R + g * 128:C_A2W2R + (g + 1) * 128],
                    h1r[:], start=True, stop=True)
                rr = sing.tile([128, 1], f32, tag=f"rr{g}")
                nc.scalar.activation(out=rr[:], in_=rps[:], func=AF.Identity,
                                     bias=wbt[:, C_A2B2R + g:C_A2B2R + g + 1],
                                     scale=1.0)
                rrep.append(rr)

            # attention3 bias: bias3 = relu(g @ a3w1.T / P + b1) @ a3w2.T + b2
            h3ps = psm.tile([COUT, 1], f32, tag="mlp")
            nc.tensor.matmul(h3ps[:], wbt[0:64, C_A3W1:C_A3W1 + 64], gsum[:],
                             start=True, stop=True)
            h3r = sing.tile([COUT, 1], f32)
            nc.scalar.activation(out=h3r[:], in_=h3ps[:], func=AF.Relu,
                                 bias=wbt[0:64, C_A3B1:C_A3B1 + 1], scale=1.0 / P)
            b3ps = psm.tile([COUT, 1], f32, tag="mlp")
            nc.tensor.matmul(b3ps[:], wbt[0:64, C_A3W2:C_A3W2 + 64], h3r[:],
                             start=True, stop=True)
            bias3 = sing.tile([COUT, 1], f32)
            nc.scalar.activation(out=bias3[:], in_=b3ps[:], func=AF.Identity,
                                 bias=wbt[0:64, C_A3B2:C_A3B2 + 1], scale=1.0)

            # fold ratio into conv weights: wf_g = wl_g * rrep_g  (f16)
            wf = []
            for g, (r0, r1, o, u) in enumerate(GROUPS):
                parts = 128 if r1 is not None else 64
                t = sing.tile([parts, COUT], f16, tag=f"wf{g}")
                nc.vector.tensor_scalar(t[:], wbt[0:parts, C_WL + g * 64:
                                                 C_WL + (g + 1) * 64],
                                        rrep[g][0:parts, :], None, OP.mult)
                wf.append(t)

            # main pixel-chunk loop, seam-split tiling over non-uniform R
            # chunks: chunk 0 carries the left halo, the final 2048 pixels
            # are split 1536+512 so the last dependency in the drain tail is
            # tiny. Matmul rhs reads split at chunk seams, relying on PSUM
            # has_written semantics (first writer overwrites, later parts
            # accumulate). Right-edge reads past P are dropped: their
            # attention is masked to zero (out-of-image taps).
            BND = [-GL] + [CH * (k + 1) for k in range(NCH)]
            NRC = len(BND) - 1          # 9 R chunks

            def rloc(q):
                for ci in range(NRC):
                    if q < BND[ci + 1]:
                        return ci, q - BND[ci]
                raise AssertionError(q)

            # REQ[m]: R chunk whose completion unlocks output chunk m fully
            REQ = [rloc((m + 1) * CH + 127)[0] if (m + 1) * CH + 127 < P
                   else NRC - 1 for m in range(NCH)]

            rtiles = []

            def emit_matmuls(m, req_ci):
                for half in range(2):
                    yps = psy.tile([COUT, 1024], f32)
                    parts = []      # (s, ci, col, pcol, w, g)
                    for g, (r0, r1, o, u) in enumerate(GROUPS):
                        for s in range(2):
                            q0 = m * CH + half * 1024 + s * 512 + o
                            left = 512
                            pcol = s * 512
                            while left > 0:
                                if q0 >= P:
                                    break   # masked-zero region
                                ci, c1 = rloc(q0)
                                w = min(left, BND[ci + 1] - q0)
                                parts.append((s, ci, c1, pcol, w, g))
                                q0 += w
                                pcol += w
                                left -= w
                    # defer parts needing the just-built newest chunk so the
                    # rest can run while its inputs are still in flight
                    parts = ([p for p in parts if p[1] < req_ci] +
                             [p for p in parts if p[1] >= req_ci])
                    first = {}
                    lastp = {}
                    for i, p in enumerate(parts):
                        first.setdefault(p[0], i)
                        lastp[p[0]] = i
                    for i, (s, ci, c1, pcol, w, g) in enumerate(parts):
                        nc.tensor.matmul(
                            yps[:, pcol:pcol + w], wf[g][:],
                            rtiles[ci][g][:, c1:c1 + w],
                            start=first[s] == i, stop=lastp[s] == i)
                    off = m * CH + half * 1024
                    nc.scalar.activation(out=Y16[:, off:off + 1024], in_=yps[:],
                                         func=AF.Identity, bias=bias3[:],
                                         scale=1.0)

            dmai = 0
            for ci in range(NRC):
                qlo, qhi = BND[ci], BND[ci + 1]
                wk = qhi - qlo
                col0 = qlo + GUARD          # XXL column of the chunk start

                # attention replica tiles (DRAM->SBUF broadcast DMA)
                arts = []
                for g, (r0, r1, o, u) in enumerate(GROUPS):
                    s0 = qlo - o + GUARD
                    if g == 3:
                        # one DMA: both halves column-concatenated (base
                        # partition 0 for both TT reads)
                        a01 = arp.tile([64, 2 * wk], f16, tag="ar3")
                        row = ag[r0:r0 + 1, s0:s0 + wk]
                        src = bass.AP(tensor=row.tensor, offset=row.offset,
                                      ap=[[0, 64], [WX, 2], list(row.ap)[-1]])
                        nc.sync.dma_start(out=a01[:], in_=src)
                        arts.append(a01)
                        continue
                    if r1 is None:
                        ar8 = arp.tile([64, wk], f16, tag="ar4")
                        row = ag[r0:r0 + 1, s0:s0 + wk]
                        src = bass.AP(tensor=row.tensor, offset=row.offset,
                                      ap=[[0, 64], list(row.ap)[-1]])
                        nc.sync.dma_start(out=ar8[:], in_=src)
                        arts.append(ar8)
                        continue
                    art = arp.tile([128, wk], f16, tag=f"ar{g}")
                    row = ag[r0:r0 + 1, s0:s0 + wk]
                    src = bass.AP(tensor=row.tensor, offset=row.offset,
                                  ap=[[WX, 2], [0, 64], list(row.ap)[-1]])
                    nc.sync.dma_start(out=art[:], in_=src)
                    arts.append(art)

                # R tiles: f16 tensor_tensor multiplies (2x mode on DVE;
                # the lone half-width tap rides the GpSimd engine)
                rts = []
                for g, (r0, r1, o, u) in enumerate(GROUPS):
                    parts_n = 128 if r1 is not None else 64
                    rt = rp.tile([parts_n, wk], f16, tag=f"r{g}")
                    if g < 3:
                        nc.vector.tensor_mul(rt[:], XXL[:, col0:col0 + wk],
                                             arts[g][:])
                    elif g == 3:
                        nc.vector.tensor_mul(rt[0:64, :],
                                             XXL[0:64, col0:col0 + wk],
                                             arts[g][:, 0:wk])
                        nc.vector.tensor_mul(rt[64:128, :],
                                             XXL[0:64, col0 + 128:col0 + 128 + wk],
                                             arts[g][:, wk:2 * wk])
                    else:
                        eng = nc.vector if ci == NRC - 1 else nc.gpsimd
                        eng.tensor_mul(rt[:], XXL[0:64, col0:col0 + wk],
                                       arts[g][:])
                    rts.append(rt)
                rtiles.append(rts)

                for m in range(NCH):
                    if REQ[m] == ci:
                        emit_matmuls(m, ci)

            # deferred output DMAs: the first piece's dependency chain
            # (evacs through chunk 6) keeps it off the fabric until the
            # attention-replica traffic has drained
            cut = (NCH - 2) * CH + 1024
            cut2 = (NCH - 1) * CH + 1024
            nc.sync.dma_start(out=y[:, 0:cut], in_=Y16[:, 0:cut])
            nc.scalar.dma_start(out=y[:, cut:cut2], in_=Y16[:, cut:cut2])
            nc.sync.dma_start(out=y[:, cut2:P], in_=Y16[:, cut2:P])
    nc.compile()
    return nc


def _host_prep(x, atw1, weight, a2_w1, a2_b1, a2_w2, a2_b2, a3_w1, a3_b1,
               a3_w2, a3_b2):
    """Build per-core input maps (host-side layout/dtype prep only)."""
    b = x.shape[0]
    f16 = np.float16

    # validity mask per permuted row: conv zero-padding baked into attention
    hh = np.arange(P) // WW
    ww = np.arange(P) % WW
    masks = np.empty((9, P), np.bool_)
    for r, t in enumerate(PERM):
        kh, kw = t // 3, t % 3
        masks[r] = ((hh + kh - 1 >= 0) & (hh + kh - 1 < HH) &
                    (ww + kw - 1 >= 0) & (ww + kw - 1 < WW))

    blob = np.zeros((128, BLOB_W), np.float32)
    for g, (r0, r1, o, u) in enumerate(GROUPS):
        t0 = PERM[r0]
        blob[0:64, C_WL + g * 64:C_WL + (g + 1) * 64] = \
            weight[:, :, t0 // 3, t0 % 3].T
        if r1 is not None:
            t1 = PERM[r1]
            blob[64:128, C_WL + g * 64:C_WL + (g + 1) * 64] = \
                weight[:, :, t1 // 3, t1 % 3].T
    blob[0:64, C_A2W1:C_A2W1 + 9] = a2_w1.T
    blob[0:9, C_A2B1] = a2_b1
    for g, (r0, r1, o, u) in enumerate(GROUPS):
        blob[0:9, C_A2W2R + g * 128:C_A2W2R + g * 128 + 64] = \
            a2_w2[PERM[r0]][:, None]
        blob[0:64, C_A2B2R + g] = a2_b2[PERM[r0]]
        if r1 is not None:
            blob[0:9, C_A2W2R + g * 128 + 64:C_A2W2R + (g + 1) * 128] = \
                a2_w2[PERM[r1]][:, None]
            blob[64:128, C_A2B2R + g] = a2_b2[PERM[r1]]
    blob[0:64, C_A3W1:C_A3W1 + 64] = a3_w1.T
    blob[0:64, C_A3B1] = a3_b1
    blob[0:64, C_A3W2:C_A3W2 + 64] = a3_w2.T
    blob[0:64, C_A3B2] = a3_b2

    selh = np.zeros((34, 128), np.float16)
    for j in range(2):
        selh[32 * j, 0:64] = 1.0
        selh[32 * j + 1, 64:128] = 1.0

    in_maps = []
    for i in range(b):
        xr = x[i].reshape(CIN, P)
        xgv = np.zeros((CIN, WX), f16)
        xgv[:, GUARD:GUARD + P] = xr.astype(f16)
        at = atw1[i].reshape(9, P)[PERM]
        agv = np.zeros((9, WX), f16)
        agv[:, GUARD:GUARD + P] = np.where(masks, at, 0.0).astype(f16)
        m = {"xg": xgv, "ag": agv, "wb": blob}
        if AR_VIA_PE:
            m["sg"] = selh
        in_maps.append(m)
    return in_maps


def kernel(**inputs):
    x = np.asarray(inputs["x"], np.float32)
    in_maps = _host_prep(
        x, np.asarray(inputs["atw1"], np.float32),
        np.asarray(inputs["weight"], np.float32),
        np.asarray(inputs["a2_w1"], np.float32),
        np.asarray(inputs["a2_b1"], np.float32),
        np.asarray(inputs["a2_w2"], np.float32),
        np.asarray(inputs["a2_b2"], np.float32),
        np.asarray(inputs["a3_w1"], np.float32),
        np.asarray(inputs["a3_b1"], np.float32),
        np.asarray(inputs["a3_w2"], np.float32),
        np.asarray(inputs["a3_b2"], np.float32),
    )
    if "nc" not in _CACHE:
        _CACHE["nc"] = _build_nc()
    nc = _CACHE["nc"]
    res = run_bass_kernel_spmd(nc, in_maps, core_ids=list(range(8)))
    out = np.stack([res.results[i]["y"].reshape(COUT, HH, WW)
                    for i in range(len(in_maps))])
    return out.astype(np.float32)


# revision 61
# speedup vs baseline: 1.0143x; 1.0073x over previous
"""Trainium2 Bass kernel for nn_AFM (attention-modulated 3x3 conv).

Math (per batch):
    ratio = MLP_a2(mean_hw(x))                       # [9]
    bias3 = MLP_a3(mean_hw(x))                       # [64]
    y[m,p] = sum_{c,t} W[m,c,t] * x[c, p+delta_t] * (atw1[t,p]*ratio[t]) + bias3[m]

Strategy: data-parallel over batch (8 cores, 1 batch each, no collectives).
Per core, fp16 compute:
  - attention (with conv zero-pad validity baked in as zeros, guard columns)
    ships as an f16 input `ag`; x ships as guarded f16 `xg`.
  - ratio is folded into the conv weights on device (so atw1 is used raw).
  - taps are processed in pairs sharing one 128-row contraction:
      R_pair[(c,t), q] = x[c, q+u_t] * ag[t, q - o_pair]
    built by DVE f16 tensor_tensor (2x mode; the lone ninth tap runs on
    GpSimd), with per-pair attention replica tiles produced by DRAM->SBUF
    broadcast DMA (partition step 0) spread over both HWDGE queues.
  - seam-split tiling: R chunks are exact-width (chunk 0 carries the left
    halo); matmul rhs reads split at chunk seams, relying on PSUM
    has_written semantics (first writer overwrites, later ones accumulate),
    so no halo columns are recomputed or re-broadcast. Out-of-range reads
    at the right edge are dropped: their attention is masked to zero.
  - 5 PSUM-accumulated f16 matmuls per 512-pixel tile compute y directly;
    ScalarE evacuates PSUM with the a3-bias add fused; y ships as f16 and
    the host upcasts.
"""

import numpy as np
from contextlib import ExitStack

import concourse.bass as bass
import concourse.tile as tile
from concourse import bacc, mybir
from concourse.bass_utils import run_bass_kernel_spmd

# permuted tap order: rows are taps [0,2, 3,5, 6,8, 1,4, 7] so that each
# matmul group's two taps sit on adjacent rows of `ag`
PERM = [0, 2, 3, 5, 6, 8, 1, 4, 7]
# groups: (row0, row1|None, o = rhs pixel offset, u = upper-half x shift)
GROUPS = [
    (0, 1, -129, 2),
    (2, 3, -1, 2),
    (4, 5, 127, 2),
    (6, 7, -128, 128),
    (8, None, 128, 0),
]
HH = 128
WW = 128
P = HH * WW           # 16384 pixels
CIN = 64
COUT = 64
GUARD = 264           # zero guard columns on xg/ag (>= 132 + 129)
GL = 132              # per-chunk halo for rhs offsets (|o| <= 129)
CH = 2048             # pixels per chunk
NCH = P // CH
WCH = CH + 2 * GL     # 2312 columns per R/ar chunk tile
WX = P + 2 * GUARD    # 16912
AR_VIA_PE = ()    # pair-groups whose attention replicas come from a
                      # PE broadcast-matmul + ScalarE evac instead of DMA

# packed small-weight blob layout (f32, [128, BLOB_W]); columns:
#   wl: 5 groups x 64      -> 0..320     (rows 0..127)
#   a2w1T [64,9]           -> 320..329   (rows 0..63)
#   a2b1 [9,1]             -> 329..330   (rows 0..8)
#   a2w2r [9,640]          -> 330..970   (rows 0..8)
#   a2b2r [128,5]          -> 970..975
#   a3w1T [64,64]          -> 975..1039  (rows 0..63)
#   a3b1 [64,1]            -> 1039..1040
#   a3w2T [64,64]          -> 1040..1104
#   a3b2 [64,1]            -> 1104..1105
BLOB_W = 1105
C_WL, C_A2W1, C_A2B1, C_A2W2R, C_A2B2R = 0, 320, 329, 330, 970
C_A3W1, C_A3B1, C_A3W2, C_A3B2 = 975, 1039, 1040, 1104

_CACHE = {}


def _build_nc():
    f32, f16 = mybir.dt.float32, mybir.dt.float16
    AF = mybir.ActivationFunctionType
    OP = mybir.AluOpType

    nc = bacc.Bacc("TRN2", target_bir_lowering=False, debug=False,
                   enable_asserts=True, num_devices=8)
    xg = nc.dram_tensor("xg", [CIN, WX], f16, kind="ExternalInput").ap()
    ag = nc.dram_tensor("ag", [9, WX], f16, kind="ExternalInput").ap()
    wb = nc.dram_tensor("wb", [128, BLOB_W], f32, kind="ExternalInput").ap()
    if AR_VIA_PE:
        sg = nc.dram_tensor("sg", [34, 128], mybir.dt.float16,
                            kind="ExternalInput").ap()
    y = nc.dram_tensor("y", [COUT, P], f16, kind="ExternalOutput").ap()

    # round-robin DMA issue engines (separate queues)
    def dq(i):
        return [nc.sync, nc.scalar][i % 2]

    with tile.TileContext(nc) as tc:
        with ExitStack() as ctx:
            sing = ctx.enter_context(tc.tile_pool(name="sing", bufs=1))
            arp = ctx.enter_context(tc.tile_pool(name="arp", bufs=2))
            rp = ctx.enter_context(tc.tile_pool(name="rp", bufs=3))
            trp = ctx.enter_context(tc.tile_pool(name="trp", bufs=1))
            psy = ctx.enter_context(tc.tile_pool(name="psy", bufs=2, space="PSUM"))
            psm = ctx.enter_context(tc.tile_pool(name="psm", bufs=2, space="PSUM"))
            psb = ctx.enter_context(tc.tile_pool(name="psb", bufs=2, space="PSUM"))

            # small-weight blob
            wbt = sing.tile([128, BLOB_W], f32)
            nc.sync.dma_start(out=wbt, in_=wb)

            # attention rows in SBUF (PE broadcast source): one tile, each
            # PE-pair's 2 rows at a legal matmul base partition (0, 32)
            atg, sel = {}, {}
            if AR_VIA_PE:
                atgt = sing.tile([34, WX], f16)
                selt = sing.tile([34, 128], f16)
                nc.sync.dma_start(out=selt, in_=sg)
                for j, g in enumerate(AR_VIA_PE):
                    r0 = GROUPS[g][0]
                    nc.scalar.dma_start(out=atgt[32 * j:32 * j + 2, :],
                                        in_=ag[r0:r0 + 2, :])
                    atg[g] = atgt[32 * j:32 * j + 2, :]
                    sel[g] = selt[32 * j:32 * j + 2, :]

            # x (f16, guarded) -> lower half of XXL, loaded in chunks; the
            # g-mean accumulation runs per chunk, alternating ScalarE (f16
            # copy with accum_out) and VectorE (reduce); the +2-shifted upper
            # half is built in per-chunk pieces so early chunks unblock fast
            XXL = sing.tile([128, WX], f16)
            gparts = sing.tile([CIN, NCH], f32)
            nc.scalar.dma_start(out=XXL[0:64, 0:GUARD], in_=xg[:, 0:GUARD])
            nc.gpsimd.dma_start(out=XXL[0:64, GUARD + P:], in_=xg[:, GUARD + P:])
            for k in range(NCH):
                c0 = GUARD + k * CH
                dq(k).dma_start(out=XXL[0:64, c0:c0 + CH], in_=xg[:, c0:c0 + CH])
                trash = trp.tile([CIN, CH], f16, tag="trash")
                nc.scalar.activation(out=trash[:], in_=XXL[0:64, c0:c0 + CH],
                                     func=AF.Copy,
                                     accum_out=gparts[:, k:k + 1])
                if k > 0:
                    w0 = (k - 1) * CH + GUARD - (GL if k == 1 else 0)
                    w1 = k * CH + GUARD
                    nc.vector.tensor_copy(out=XXL[64:128, w0:w1],
                                          in_=XXL[0:64, w0 + 2:w1 + 2])
            w0 = (NCH - 1) * CH + GUARD
            w1 = NCH * CH + GUARD
            nc.vector.tensor_copy(out=XXL[64:128, w0:w1],
                                  in_=XXL[0:64, w0 + 2:w1 + 2])

            Y16 = sing.tile([COUT, P], f16)   # staged f16 output

            gsum = sing.tile([CIN, 1], f32)
            nc.vector.tensor_reduce(gsum[:], gparts[:], axis=mybir.AxisListType.X,
                                    op=OP.add)

            # attention2: h1 = relu(g @ a2w1.T / P + b1); ratio replicated per
            # group via host-widened a2w2 columns
            h1ps = psm.tile([9, 1], f32, tag="mlp")
            nc.tensor.matmul(h1ps[:], wbt[0:64, C_A2W1:C_A2W1 + 9], gsum[:],
                             start=True, stop=True)
            h1r = sing.tile([9, 1], f32)
            nc.scalar.activation(out=h1r[:], in_=h1ps[:], func=AF.Relu,
                                 bias=wbt[0:9, C_A2B1:C_A2B1 + 1], scale=1.0 / P)
            rrep = []
            for g in range(5):
                rps = psm.tile([128, 1], f32, tag="mlp")
                nc.tensor.matmul(
                    rps[:], wbt[0:9, C_A2W2R + g * 128:C_A2W2R + (g + 1) * 128],
                    h1r[:], start=True, stop=True)
                rr = sing.tile([128, 1], f32, tag=f"rr{g}")
                nc.scalar.activation(out=rr[:], in_=rps[:], func=AF.Identity,
                                     bias=wbt[:, C_A2B2R + g:C_A2B2R + g + 1],
                                     scale=1.0)
                rrep.append(rr)

            # attention3 bias: bias3 = relu(g @ a3w1.T / P + b1) @ a3w2.T + b2
            h3ps = psm.tile([COUT, 1], f32, tag="mlp")
            nc.tensor.matmul(h3ps[:], wbt[0:64, C_A3W1:C_A3W1 + 64], gsum[:],
                             start=True, stop=True)
            h3r = sing.tile([COUT, 1], f32)
            nc.scalar.activation(out=h3r[:], in_=h3ps[:], func=AF.Relu,
                                 bias=wbt[0:64, C_A3B1:C_A3B1 + 1], scale=1.0 / P)
            b3ps = psm.tile([COUT, 1], f32, tag="mlp")
            nc.tensor.matmul(b3ps[:], wbt[0:64, C_A3W2:C_A3W2 + 64], h3r[:],
                             start=True, stop=True)
            bias3 = sing.tile([COUT, 1], f32)
            nc.scalar.activation(out=bias3[:], in_=b3ps[:], func=AF.Identity,
                                 bias=wbt[0:64, C_A3B2:C_A3B2 + 1], scale=1.0)

            # fold ratio into conv weights: wf_g = wl_g * rrep_g  (f16)
            wf = []
            for g, (r0, r1, o, u) in enumerate(GROUPS):
                parts = 128 if r1 is not None else 64
                t = sing.tile([parts, COUT], f16, tag=f"wf{g}")
                nc.vector.tensor_scalar(t[:], wbt[0:parts, C_WL + g * 64:
                                                 C_WL + (g + 1) * 64],
                                        rrep[g][0:parts, :], None, OP.mult)
                wf.append(t)

            # main pixel-chunk loop, seam-split tiling over non-uniform R
            # chunks: chunk 0 carries the left halo, the final 2048 pixels
            # are split 1536+512 so the last dependency in the drain tail is
            # tiny. Matmul rhs reads split at chunk seams, relying on PSUM
            # has_written semantics (first writer overwrites, later parts
            # accumulate). Right-edge reads past P are dropped: their
            # attention is masked to zero (out-of-image taps).
            BND = [-GL] + [CH * (k + 1) for k in range(NCH)]
            NRC = len(BND) - 1          # 9 R chunks

            def rloc(q):
                for ci in range(NRC):
                    if q < BND[ci + 1]:
                        return ci, q - BND[ci]
                raise AssertionError(q)

            # REQ[m]: R chunk whose completion unlocks output chunk m fully
            REQ = [rloc((m + 1) * CH + 127)[0] if (m + 1) * CH + 127 < P
                   else NRC - 1 for m in range(NCH)]

            rtiles = []

            def emit_matmuls(m, req_ci):
                for half in range(2):
                    yps = psy.tile([COUT, 1024], f32)
                    parts = []      # (s, ci, col, pcol, w, g)
                    for g, (r0, r1, o, u) in enumerate(GROUPS):
                        for s in range(2):
                            q0 = m * CH + half * 1024 + s * 512 + o
                            left = 512
                            pcol = s * 512
                            while left > 0:
                                if q0 >= P:
                                    break   # masked-zero region
                                ci, c1 = rloc(q0)
                                w = min(left, BND[ci + 1] - q0)
                                parts.append((s, ci, c1, pcol, w, g))
                                q0 += w
                                pcol += w
                                left -= w
                    # defer parts needing the just-built newest chunk so the
                    # rest can run while its inputs are still in flight
                    parts = ([p for p in parts if p[1] < req_ci] +
                             [p for p in parts if p[1] >= req_ci])
                    first = {}
                    lastp = {}
                    for i, p in enumerate(parts):
                        first.setdefault(p[0], i)
                        lastp[p[0]] = i
                    for i, (s, ci, c1, pcol, w, g) in enumerate(parts):
                        nc.tensor.matmul(
                            yps[:, pcol:pcol + w], wf[g][:],
                            rtiles[ci][g][:, c1:c1 + w],
                            start=first[s] == i, stop=lastp[s] == i)
                    off = m * CH + half * 1024
                    nc.scalar.activation(out=Y16[:, off:off + 1024], in_=yps[:],
                                         func=AF.Identity, bias=bias3[:],
                                         scale=1.0)

            dmai = 0
            for ci in range(NRC):
                qlo, qhi = BND[ci], BND[ci + 1]
                wk = qhi - qlo
                col0 = qlo + GUARD          # XXL column of the chunk start

                # attention replica tiles (DRAM->SBUF broadcast DMA)
                arts = []
                for g, (r0, r1, o, u) in enumerate(GROUPS):
                    s0 = qlo - o + GUARD
                    if g == 3:
                        # one DMA: both halves column-concatenated (base
                        # partition 0 for both TT reads)
                        a01 = arp.tile([64, 2 * wk], f16, tag="ar3")
                        row = ag[r0:r0 + 1, s0:s0 + wk]
                        src = bass.AP(tensor=row.tensor, offset=row.offset,
                                      ap=[[0, 64], [WX, 2], list(row.ap)[-1]])
                        nc.sync.dma_start(out=a01[:], in_=src)
                        arts.append(a01)
                        continue
                    if r1 is None:
                        ar8 = arp.tile([64, wk], f16, tag="ar4")
                        row = ag[r0:r0 + 1, s0:s0 + wk]
                        src = bass.AP(tensor=row.tensor, offset=row.offset,
                                      ap=[[0, 64], list(row.ap)[-1]])
                        nc.sync.dma_start(out=ar8[:], in_=src)
                        arts.append(ar8)
                        continue
                    art = arp.tile([128, wk], f16, tag=f"ar{g}")
                    row = ag[r0:r0 + 1, s0:s0 + wk]
                    src = bass.AP(tensor=row.tensor, offset=row.offset,
                                  ap=[[WX, 2], [0, 64], list(row.ap)[-1]])
                    nc.sync.dma_start(out=art[:], in_=src)
                    arts.append(art)

                # R tiles: f16 tensor_tensor multiplies (2x mode on DVE;
                # the lone half-width tap rides the GpSimd engine)
                rts = []
                for g, (r0, r1, o, u) in enumerate(GROUPS):
                    parts_n = 128 if r1 is not None else 64
                    rt = rp.tile([parts_n, wk], f16, tag=f"r{g}")
                    if g < 3:
                        nc.vector.tensor_mul(rt[:], XXL[:, col0:col0 + wk],
                                             arts[g][:])
                    elif g == 3:
                        nc.vector.tensor_mul(rt[0:64, :],
                                             XXL[0:64, col0:col0 + wk],
                                             arts[g][:, 0:wk])
                        nc.vector.tensor_mul(rt[64:128, :],
                                             XXL[0:64, col0 + 128:col0 + 128 + wk],
                                             arts[g][:, wk:2 * wk])
                    else:
                        eng = nc.vector if ci == NRC - 1 else nc.gpsimd
                        eng.tensor_mul(rt[:], XXL[0:64, col0:col0 + wk],
                                       arts[g][:])
                    rts.append(rt)
                rtiles.append(rts)

                for m in range(NCH):
                    if REQ[m] == ci:
                        emit_matmuls(m, ci)

            # deferred output DMAs: the first piece's dependency chain
            # (evacs through chunk 6) keeps it off the fabric until the
            # attention-replica traffic has drained
            cut = (NCH - 2) * CH + 1024
            cut2 = (NCH - 1) * CH + 1024
            nc.sync.dma_start(out=y[:, 0:cut], in_=Y16[:, 0:cut])
            nc.scalar.dma_start(out=y[:, cut:cut2], in_=Y16[:, cut:cut2])
            nc.sync.dma_start(out=y[:, cut2:P], in_=Y16[:, cut2:P])
    nc.compile()
    return nc


def _host_prep(x, atw1, weight, a2_w1, a2_b1, a2_w2, a2_b2, a3_w1, a3_b1,
               a3_w2, a3_b2):
    """Build per-core input maps (host-side layout/dtype prep only)."""
    b = x.shape[0]
    f16 = np.float16

    # validity mask per permuted row: conv zero-padding baked into attention
    hh = np.arange(P) // WW
    ww = np.arange(P) % WW
    masks = np.empty((9, P), np.bool_)
    for r, t in enumerate(PERM):
        kh, kw = t // 3, t % 3
        masks[r] = ((hh + kh - 1 >= 0) & (hh + kh - 1 < HH) &
                    (ww + kw - 1 >= 0) & (ww + kw - 1 < WW))

    blob = np.zeros((128, BLOB_W), np.float32)
    for g, (r0, r1, o, u) in enumerate(GROUPS):
        t0 = PERM[r0]
        blob[0:64, C_WL + g * 64:C_WL + (g + 1) * 64] = \
            weight[:, :, t0 // 3, t0 % 3].T
        if r1 is not None:
            t1 = PERM[r1]
            blob[64:128, C_WL + g * 64:C_WL + (g + 1) * 64] = \
                weight[:, :, t1 // 3, t1 % 3].T
    blob[0:64, C_A2W1:C_A2W1 + 9] = a2_w1.T
    blob[0:9, C_A2B1] = a2_b1
    for g, (r0, r1, o, u) in enumerate(GROUPS):
        blob[0:9, C_A2W2R + g * 128:C_A2W2R + g * 128 + 64] = \
            a2_w2[PERM[r0]][:, None]
        blob[0:64, C_A2B2R + g] = a2_b2[PERM[r0]]
        if r1 is not None:
            blob[0:9, C_A2W2R + g * 128 + 64:C_A2W2R + (g + 1) * 128] = \
                a2_w2[PERM[r1]][:, None]
            blob[64:128, C_A2B2R + g] = a2_b2[PERM[r1]]
    blob[0:64, C_A3W1:C_A3W1 + 64] = a3_w1.T
    blob[0:64, C_A3B1] = a3_b1
    blob[0:64, C_A3W2:C_A3W2 + 64] = a3_w2.T
    blob[0:64, C_A3B2] = a3_b2

    selh = np.zeros((34, 128), np.float16)
    for j in range(2):
        selh[32 * j, 0:64] = 1.0
        selh[32 * j + 1, 64:128] = 1.0

    in_maps = []
    for i in range(b):
        xr = x[i].reshape(CIN, P)
        xgv = np.zeros((CIN, WX), f16)
        xgv[:, GUARD:GUARD + P] = xr.astype(f16)
        at = atw1[i].reshape(9, P)[PERM]
        agv = np.zeros((9, WX), f16)
        agv[:, GUARD:GUARD + P] = np.where(masks, at, 0.0).astype(f16)
        m = {"xg": xgv, "ag": agv, "wb": blob}
        if AR_VIA_PE:
            m["sg"] = selh
        in_maps.append(m)
    return in_maps


def kernel(**inputs):
    x = np.asarray(inputs["x"], np.float32)
    in_maps = _host_prep(
        x, np.asarray(inputs["atw1"], np.float32),
        np.asarray(inputs["weight"], np.float32),
        np.asarray(inputs["a2_w1"], np.float32),
        np.asarray(inputs["a2_b1"], np.float32),
        np.asarray(inputs["a2_w2"], np.float32),
        np.asarray(inputs["a2_b2"], np.float32),
        np.asarray(inputs["a3_w1"], np.float32),
        np.asarray(inputs["a3_b1"], np.float32),
        np.asarray(inputs["a3_w2"], np.float32),
        np.asarray(inputs["a3_b2"], np.float32),
    )
    if "nc" not in _CACHE:
        _CACHE["nc"] = _build_nc()
    nc = _CACHE["nc"]
    res = run_bass_kernel_spmd(nc, in_maps, core_ids=list(range(8)))
    out = np.stack([res.results[i]["y"].reshape(COUT, HH, WW)
                    for i in range(len(in_maps))])
    return out.astype(np.float32)
